# revision 10
# baseline (speedup 1.0000x reference)
"""BertAttention (feature-axis scores) Trainium2 Bass kernel, 8-core SPMD.

Reference computation (S=16384, D=1024, fp32):
    q = x @ Wq.T + bq ; k = x @ Wk.T + bk ; v = x @ Wv.T + bv
    scores = einsum('sd,se->de', q, k) / sqrt(D)        # [D, D]
    attn = softmax(scores, axis=-1)
    out = v @ attn                                       # [S, D]
    return out, attn

Restructuring (biases folded via augmented matrices):
    x~ = [x | 1 | 0]                  [S, D+2]  (zero col keeps f32r N even)
    W~q = [Wq | bq | 0], W~k likewise
    G~ = x~^T x~                      [D+2, D+2]  (seq contraction -> no x transpose)
    scores = W~q G~ W~k^T / sqrt(D)
    attn = softmax_rows(scores)
    out = x~ @ (W~v^T attn)   with W~v = [Wv | bv] -> last row = bv^T attn

G~ is symmetric: only upper-triangle column blocks are computed and
AllReduced (packed, split into two collectives so the first half of the
reduce overlaps the second half of the compute); mirror blocks are
reconstructed on-chip by PE transpose.

The scores/attn path runs in float32r (full fp32 inputs, ~1e-4 matmul
rounding) so the attn output stays accurate.  The post-softmax path
(attn AllGather, W2 = W~v^T attn, out = x~ @ W2~) runs in bf16, which
halves the gather payload and the x^T SBUF footprint.

Sharding: core i owns x rows [2048*i, 2048*(i+1)) and scores/attn rows
[128*i, 128*(i+1)).
"""
import sys
import numpy as np
import ml_dtypes

if "/opt/trn_rl_repo" not in sys.path:
    sys.path.insert(0, "/opt/trn_rl_repo")

import concourse.bass as bass
import concourse.mybir as mybir
import concourse.tile as tile
from concourse import bacc
from concourse.bass_utils import run_bass_kernel_spmd


def _install_ntff_hook_shim():
    """The agent image's antenv lacks axon_hooks; provide it so trace=True
    (NTFF profiling) works through run_bass_kernel_spmd."""
    import types
    if "antenv.axon_hooks" in sys.modules:
        return
    mod = types.ModuleType("antenv.axon_hooks")
    mod._hook = None
    mod.set_axon_ntff_profile_hook = lambda h: setattr(mod, "_hook", h)
    mod.get_axon_ntff_profile_hook = lambda: mod._hook
    sys.modules["antenv.axon_hooks"] = mod
    try:
        sys.path.insert(0, "/root/.axon_site")
        from trn_agent_boot.trn_boot import _ntff_profile_via_ctypes
        mod._hook = _ntff_profile_via_ctypes("/opt/axon/libaxon_pjrt.so")
    except Exception:
        pass


_install_ntff_hook_shim()

N_CORES = 8
S, D = 16384, 1024
S_SH = S // N_CORES          # 2048 seq rows per core
P = 128                      # partitions
NS = S_SH // P               # 16 seq chunks per core
ND = D // P                  # 8 feature chunks
H = 512                      # matmul free-dim half
DA = D + 2                   # augmented dim
f32 = mybir.dt.float32
f32r = mybir.dt.float32r
bf16 = mybir.dt.bfloat16
AX = mybir.AxisListType.X
ADD = mybir.AluOpType.add
MAX = mybir.AluOpType.max
EXP = mybir.ActivationFunctionType.Exp
SCALE = 1.0 / np.sqrt(np.float32(D))   # 1/32

# symmetric-G packing: per 128-row block bt, the column groups in
# GBLK[bt] (col0, width) are computed/shipped; the rest is mirrored.
GBLK = {
    0: [(0, 512), (512, 512)],
    1: [(0, 512), (512, 512)],
    2: [(256, 256), (512, 512)],
    3: [(256, 256), (512, 512)],
    4: [(512, 512)],
    5: [(512, 512)],
    6: [(768, 256)],
    7: [(768, 256)],
}
C0 = [blks[0][0] for bt, blks in sorted(GBLK.items())]
WID = [D - c for c in C0]
OFF = np.cumsum([0] + [P * w for w in WID]).tolist()
SIZE_A = OFF[4]                       # packed bytes (floats) of bt 0..3
AUG_OFF_B = OFF[8] - SIZE_A           # aug row offset inside buffer B
SIZE_B = AUG_OFF_B + 2 * DA


def build_program():
    nc = bacc.Bacc("TRN2", target_bir_lowering=False, debug=False,
                   num_devices=N_CORES)

    x_sh = nc.dram_tensor("x_sh", [S_SH, D], f32r, kind="ExternalInput").ap()
    wq_sh = nc.dram_tensor("wq_sh", [P, D], f32r, kind="ExternalInput").ap()
    bq_row = nc.dram_tensor("bq_row", [2, P], f32r, kind="ExternalInput").ap()
    wk = nc.dram_tensor("wk", [D, D], f32r, kind="ExternalInput").ap()
    bk_row = nc.dram_tensor("bk_row", [2, D], f32r, kind="ExternalInput").ap()
    wv_b = nc.dram_tensor("wv_b", [D, D], bf16, kind="ExternalInput").ap()
    bv_cols = nc.dram_tensor("bv_cols", [P, ND], bf16, kind="ExternalInput").ap()
    ident_d = nc.dram_tensor("ident", [P, P], f32r, kind="ExternalInput").ap()
    aug_cols_d = nc.dram_tensor("aug_cols", [P, 2], f32r, kind="ExternalInput").ap()
    ones_row_d = nc.dram_tensor("ones_row", [1, P], bf16, kind="ExternalInput").ap()

    out_sh = nc.dram_tensor("out_sh", [S_SH, D], f32, kind="ExternalOutput").ap()
    attn_sh = nc.dram_tensor("attn_sh", [P, D], f32, kind="ExternalOutput").ap()

    RG = [list(range(N_CORES))]

    with tile.TileContext(nc) as tc:
        with tc.tile_pool(name="misc", bufs=1) as misc, \
             tc.tile_pool(name="dram", bufs=1, space="DRAM") as dram:
            ident = misc.tile([P, P], f32r)
            aug_cols = misc.tile([P, 2], f32r)
            ones_row = misc.tile([1, P], bf16)
            bq_sb = misc.tile([2, P], f32r)
            bk_sb = misc.tile([2, D], f32r)
            nc.sync.dma_start(ident[:], ident_d[:])
            nc.sync.dma_start(aug_cols[:], aug_cols_d[:])
            nc.sync.dma_start(ones_row[:], ones_row_d[:])
            nc.sync.dma_start(bq_sb[:], bq_row[:])
            nc.sync.dma_start(bk_sb[:], bk_row[:])
            # touch the ACT exp table early so its load is off the
            # softmax critical path
            warm = misc.tile([1, 2], f32)
            nc.any.memset(warm[:], 0.0)
            nc.scalar.activation(warm[:], warm[:], EXP)

            gar_a = dram.tile([SIZE_A], f32)
            gar_ao = dram.tile([SIZE_A], f32, addr_space="Shared")
            gar_b = dram.tile([SIZE_B], f32)
            gar_bo = dram.tile([SIZE_B], f32, addr_space="Shared")
            ag_in = [dram.tile([P, H], bf16, name=f"agi{h}") for h in range(2)]
            ag_out = [dram.tile([D, H], bf16, name=f"ago{h}",
                                addr_space="Shared") for h in range(2)]

            def gpack(buf, bt, c0, w):
                base = OFF[bt] if bt < 4 else OFF[bt] - SIZE_A
                sl = buf[base:base + P * WID[bt]] \
                    .rearrange("(p w) -> p w", w=WID[bt])
                return sl[:, c0 - C0[bt]:c0 - C0[bt] + w]

            with tc.tile_pool(name="wv_pool", bufs=1) as wv_pool, \
                 tc.tile_pool(name="xT_pool", bufs=1) as xT_pool:
                wv_sb = wv_pool.tile([P, ND, D], bf16)
                bv_sb = wv_pool.tile([P, ND], bf16)
                xT = xT_pool.tile([P, ND, S_SH], bf16)   # x^T[e, s]

                with tc.tile_pool(name="xhi_pool", bufs=1) as xhi_pool:
                    x_hi = xhi_pool.tile([P, NS // 2, D], f32r)

                    # ============ Phase 1 ============
                    with tc.tile_pool(name="wkT_pool", bufs=1) as wkT_pool, \
                         tc.tile_pool(name="wq_pool", bufs=1) as wq_pool:
                        wkT = wkT_pool.tile([P, ND, D], f32r)   # Wk^T[b, e]
                        wq_sb = wq_pool.tile([P, D], f32r)
                        wqT = wq_pool.tile([P, ND, P], f32r)    # Wq_i^T[a, d]

                        with tc.tile_pool(name="xlo_pool", bufs=1) as xlo_pool:
                            x_lo = xlo_pool.tile([P, NS // 2, D], f32r)
                            xr = x_sh.rearrange("(n p) e -> p n e", p=P)

                            def xk(ks):
                                return (x_lo[:, ks, :] if ks < NS // 2
                                        else x_hi[:, ks - NS // 2, :])

                            for ks in range(NS):
                                eng = nc.sync if ks % 2 == 0 else nc.scalar
                                eng.dma_start(xk(ks), xr[:, ks, :])
                            nc.sync.dma_start(wq_sb[:], wq_sh[:])
                            nc.sync.dma_start(
                                wv_sb[:],
                                wv_b.rearrange("(ct p) e -> p ct e", p=P))
                            nc.sync.dma_start(bv_sb[:], bv_cols[:])

                            # ---- G~ upper blocks ----
                            with tc.tile_pool(name="gstage", bufs=3) as gstage, \
                                 tc.tile_pool(name="psum_g5", bufs=4,
                                              space="PSUM") as psum_g5, \
                                 tc.tile_pool(name="psum_g2", bufs=2,
                                              space="PSUM") as psum_g2, \
                                 tc.tile_pool(name="psum_ga", bufs=2,
                                              space="PSUM") as psum_ga:
                                def g_block(bt, buf):
                                    pss = []
                                    for (c0, w) in GBLK[bt]:
                                        pool = psum_g5 if w == H else psum_g2
                                        pss.append(pool.tile(
                                            [P, w], f32, name="gps",
                                            tag=f"g{w}"))
                                    for ks in range(NS):
                                        for gi, (c0, w) in enumerate(GBLK[bt]):
                                            nc.tensor.matmul(
                                                pss[gi][:],
                                                xk(ks)[:, bt * P:(bt + 1) * P],
                                                xk(ks)[:, c0:c0 + w],
                                                start=(ks == 0),
                                                stop=(ks == NS - 1))
                                    for gi, (c0, w) in enumerate(GBLK[bt]):
                                        st = gstage.tile([P, H], f32,
                                                         name="gst", tag="gst")
                                        nc.vector.tensor_copy(st[:, :w], pss[gi][:])
                                        nc.scalar.dma_start(
                                            gpack(buf, bt, c0, w), st[:, :w])

                                for bt in range(4):
                                    g_block(bt, gar_a)

                                # ---- AllReduce part A (bt 0..3) ----
                                nc.gpsimd.collective_compute(
                                    "AllReduce", ADD, replica_groups=RG,
                                    ins=[gar_a[:]], outs=[gar_ao[:]],
                                )

                                for bt in range(4, ND):
                                    g_block(bt, gar_b)

                                # aug row [2, DA] = [sx^T, S, 0; 0...]
                                stage_a = gstage.tile([2, DA], f32, name="gsta",
                                                      tag="gst")
                                for nh in range(3):
                                    n0, n1 = ((nh * H, (nh + 1) * H) if nh < 2
                                              else (D, DA))
                                    n = n1 - n0
                                    ps = psum_ga.tile([P, H], f32, name="gpa",
                                                      tag="gpa")
                                    for ks in range(NS):
                                        rhs = (xk(ks)[:, n0:n1] if nh < 2
                                               else aug_cols[:])
                                        nc.tensor.matmul(ps[:2, :n], aug_cols[:],
                                                         rhs,
                                                         start=(ks == 0),
                                                         stop=(ks == NS - 1))
                                    nc.vector.tensor_copy(stage_a[:, n0:n1],
                                                       ps[:2, :n])
                                nc.scalar.dma_start(
                                    gar_b[AUG_OFF_B:AUG_OFF_B + 2 * DA]
                                    .rearrange("(p w) -> p w", w=DA),
                                    stage_a[:])

                            # ---- AllReduce part B (bt 4..7 + aug) ----
                            ar2_inst = nc.gpsimd.collective_compute(
                                "AllReduce", ADD, replica_groups=RG,
                                ins=[gar_b[:]], outs=[gar_bo[:]],
                            )

                            # ---- under AR: transpose x (ks 0..7), Wk, Wq_i
                            with tc.tile_pool(name="wk_chunk", bufs=2) as wk_chunk, \
                                 tc.tile_pool(name="psum_t", bufs=4,
                                              space="PSUM") as psum_t:
                                for ec in range(ND):
                                    for ks in range(NS // 2):
                                        pt = psum_t.tile([P, P], f32r,
                                                         name="pt", tag="pt")
                                        ti = nc.tensor.transpose(
                                            pt[:],
                                            xk(ks)[:, ec * P:(ec + 1) * P],
                                            ident[:])
                                        tile.add_dep_helper(
                                            ar2_inst.ins, ti.ins, False,
                                            "keep PE on G until AR2 queued")
                                        nc.vector.tensor_copy(
                                            xT[:, ec, ks * P:(ks + 1) * P],
                                            pt[:])

                                for eb in range(ND):
                                    wkc = wk_chunk.tile([P, D], f32r,
                                                        name="wkc")
                                    nc.sync.dma_start(
                                        wkc[:], wk[eb * P:(eb + 1) * P, :])
                                    for bt in range(ND):
                                        pt = psum_t.tile([P, P], f32r,
                                                         name="pt2", tag="pt")
                                        ti = nc.tensor.transpose(
                                            pt[:], wkc[:, bt * P:(bt + 1) * P],
                                            ident[:])
                                        tile.add_dep_helper(
                                            ar2_inst.ins, ti.ins, False,
                                            "keep PE on G until AR2 queued")
                                        nc.vector.tensor_copy(
                                            wkT[:, bt, eb * P:(eb + 1) * P],
                                            pt[:])

                                for at in range(ND):
                                    pt = psum_t.tile([P, P], f32r, name="pt3",
                                                     tag="pt")
                                    nc.tensor.transpose(
                                        pt[:], wq_sb[:, at * P:(at + 1) * P],
                                        ident[:])
                                    nc.vector.tensor_copy(wqT[:, at, :], pt[:])
                        # x_lo freed

                        # ---- unpack G~ + mirrors + A~ + scores + softmax ----
                        with tc.tile_pool(name="g_pool", bufs=1) as g_pool, \
                             tc.tile_pool(name="sm_pool", bufs=1) as sm_pool, \
                             tc.tile_pool(name="psum_t2", bufs=4,
                                          space="PSUM") as psum_t2, \
                             tc.tile_pool(name="psum_a", bufs=2,
                                          space="PSUM") as psum_a, \
                             tc.tile_pool(name="psum_s", bufs=1,
                                          space="PSUM") as psum_s:
                            gsb = g_pool.tile([P, ND, DA], f32r)
                            grow = g_pool.tile([2, DA], f32r)
                            for bt in range(ND):
                                src = gpack(gar_ao if bt < 4 else gar_bo,
                                            bt, C0[bt], WID[bt])
                                nc.scalar.dma_start(gsb[:, bt, C0[bt]:D],
                                                    src.bitcast(f32r))
                            nc.scalar.dma_start(
                                grow[:],
                                gar_bo[AUG_OFF_B:AUG_OFF_B + 2 * DA]
                                .rearrange("(p w) -> p w", w=DA).bitcast(f32r))
                            # mirror lower blocks: G[bt, cb] = G[cb, bt]^T
                            for bt in range(ND):
                                for cb in range(C0[bt] // P):
                                    pt = psum_t2.tile([P, P], f32r, name="mir",
                                                      tag="mir")
                                    nc.tensor.transpose(
                                        pt[:], gsb[:, cb, bt * P:(bt + 1) * P],
                                        ident[:])
                                    nc.vector.tensor_copy(
                                        gsb[:, bt, cb * P:(cb + 1) * P], pt[:])
                            # aug cols: G[bt, 1024:1026] = grow[:, bt-range]^T
                            for bt in range(ND):
                                pt = psum_t2.tile([P, P], f32r, name="mira",
                                                  tag="mir")
                                nc.tensor.transpose(
                                    pt[:, :2], grow[:2, bt * P:(bt + 1) * P],
                                    ident[:2, :2])
                                nc.vector.tensor_copy(gsb[:, bt, D:DA], pt[:, :2])

                            # A~ = W~q_i G~
                            A_sb = g_pool.tile([P, DA], f32r)
                            for nh in range(3):
                                n0, n1 = ((nh * H, (nh + 1) * H) if nh < 2
                                          else (D, DA))
                                n = n1 - n0
                                ps = psum_a.tile([P, H], f32, name="aps",
                                                 tag="aps")
                                for ac in range(ND):
                                    nc.tensor.matmul(ps[:, :n], wqT[:, ac, :],
                                                     gsb[:, ac, n0:n1],
                                                     start=(ac == 0),
                                                     stop=False)
                                nc.tensor.matmul(ps[:, :n], bq_sb[:],
                                                 grow[:, n0:n1],
                                                 start=False, stop=True)
                                nc.vector.tensor_copy(A_sb[:, n0:n1], ps[:, :n])

                            AT = g_pool.tile([P, ND, P], f32r)
                            A_last = g_pool.tile([2, P], f32r)
                            for bc in range(ND):
                                pt = psum_t2.tile([P, P], f32r, name="at",
                                                  tag="mir")
                                nc.tensor.transpose(
                                    pt[:], A_sb[:, bc * P:(bc + 1) * P],
                                    ident[:])
                                nc.vector.tensor_copy(AT[:, bc, :], pt[:])
                            pt = psum_t2.tile([P, P], f32r, name="at2",
                                              tag="mir")
                            nc.tensor.transpose(pt[:2, :], A_sb[:, D:DA],
                                                ident[:])
                            nc.vector.tensor_copy(A_last[:], pt[:2, :])

                            # scores in one 2-bank PSUM tile for 1-pass softmax
                            scp = psum_s.tile([P, D], f32, name="scp")
                            for nh in range(2):
                                for bc in range(ND):
                                    nc.tensor.matmul(
                                        scp[:, nh * H:(nh + 1) * H],
                                        AT[:, bc, :],
                                        wkT[:, bc, nh * H:(nh + 1) * H],
                                        start=(bc == 0), stop=False)
                                nc.tensor.matmul(scp[:, nh * H:(nh + 1) * H],
                                                 A_last[:],
                                                 bk_sb[:, nh * H:(nh + 1) * H],
                                                 start=False, stop=True)

                            mxc = sm_pool.tile([P, 1], f32)
                            nc.vector.reduce_max(mxc[:], scp[:], axis=AX)
                            negm = sm_pool.tile([P, 1], f32)
                            nc.vector.tensor_scalar_mul(negm[:], mxc[:],
                                                        -float(SCALE))
                            tsum = sm_pool.tile([P, 1], f32)
                            attn_sb = sm_pool.tile([P, D], f32)
                            nc.scalar.activation(
                                attn_sb[:], scp[:], EXP, bias=negm[:, 0:1],
                                scale=float(SCALE),
                                accum_out=tsum[:, 0:1])
                            rinv = sm_pool.tile([P, 1], f32)
                            nc.vector.reciprocal(rinv[:], tsum[:])
                            nc.vector.tensor_scalar_mul(attn_sb[:], attn_sb[:],
                                                        rinv[:, 0:1])

                            nc.sync.dma_start(attn_sh[:], attn_sb[:])
                            attn_bf = sm_pool.tile([P, D], bf16)
                            nc.vector.tensor_copy(attn_bf[:], attn_sb[:])
                            for h in range(2):
                                nc.scalar.dma_start(
                                    ag_in[h][:],
                                    attn_bf[:, h * H:(h + 1) * H])

                    # ---- AllGather attn rows (bf16, split by col half) ----
                    for h in range(2):
                        nc.gpsimd.collective_compute(
                            "AllGather", mybir.AluOpType.bypass,
                            replica_groups=RG,
                            ins=[ag_in[h][:]], outs=[ag_out[h][:]],
                        )

                    # ---- under AG: transpose x (ks 8..15) ----
                    with tc.tile_pool(name="psum_t3", bufs=4,
                                      space="PSUM") as psum_t3:
                        for ec in range(ND):
                            for ks in range(NS // 2, NS):
                                pt = psum_t3.tile([P, P], f32r, name="ptl",
                                                  tag="ptl")
                                nc.tensor.transpose(
                                    pt[:],
                                    x_hi[:, ks - NS // 2,
                                         ec * P:(ec + 1) * P],
                                    ident[:])
                                nc.vector.tensor_copy(
                                    xT[:, ec, ks * P:(ks + 1) * P], pt[:])
                # x_hi freed

                # ======== Phase 2: W2~ = W~v^T attn ; out = x~ @ W2~ ========
                with tc.tile_pool(name="w2_pool", bufs=1) as w2_pool:
                    w2 = w2_pool.tile([P, ND, D], bf16)
                    w2row = w2_pool.tile([1, D], bf16)

                    with tc.tile_pool(name="aa_pool", bufs=1) as aa_pool, \
                         tc.tile_pool(name="psum_w", bufs=4,
                                      space="PSUM") as psum_w:
                        attn_all = [aa_pool.tile([P, ND, H], bf16,
                                                 name=f"aa{h}")
                                    for h in range(2)]
                        for h in range(2):
                            agr = ag_out[h][:].rearrange(
                                "(ct p) d -> p ct d", p=P)
                            nc.scalar.dma_start(attn_all[h][:], agr[:])

                        for nh in range(2):
                            for et in range(ND):
                                ps = psum_w.tile([P, H], f32, name="wps",
                                                 tag="wps")
                                for ct in range(ND):
                                    nc.tensor.matmul(
                                        ps[:],
                                        wv_sb[:, ct, et * P:(et + 1) * P],
                                        attn_all[nh][:, ct, :],
                                        start=(ct == 0), stop=(ct == ND - 1))
                                nc.vector.tensor_copy(
                                    w2[:, et, nh * H:(nh + 1) * H], ps[:])
                            ps = psum_w.tile([P, H], f32, name="wps2",
                                             tag="wps")
                            for ct in range(ND):
                                nc.tensor.matmul(
                                    ps[:1, :], bv_sb[:, ct:ct + 1],
                                    attn_all[nh][:, ct, :],
                                    start=(ct == 0), stop=(ct == ND - 1))
                            nc.vector.tensor_copy(w2row[:, nh * H:(nh + 1) * H],
                                               ps[:1, :])

                    with tc.tile_pool(name="o_pool", bufs=4) as o_pool, \
                         tc.tile_pool(name="psum_o", bufs=4,
                                      space="PSUM") as psum_o:
                        for nh in range(2):
                            for st in range(NS):
                                ps = psum_o.tile([P, H], f32, name="ops",
                                                 tag="ops")
                                for ec in range(ND):
                                    nc.tensor.matmul(
                                        ps[:],
                                        xT[:, ec, st * P:(st + 1) * P],
                                        w2[:, ec, nh * H:(nh + 1) * H],
                                        start=(ec == 0), stop=False)
                                nc.tensor.matmul(ps[:], ones_row[:],
                                                 w2row[:, nh * H:(nh + 1) * H],
                                                 start=False, stop=True)
                                ost = o_pool.tile([P, H], f32, name="ost")
                                nc.vector.tensor_copy(ost[:], ps[:])
                                nc.sync.dma_start(
                                    out_sh[st * P:(st + 1) * P,
                                           nh * H:(nh + 1) * H], ost[:])

    nc.compile()
    return nc


_NC_CACHE = {}


def _get_program():
    if "nc" not in _NC_CACHE:
        _NC_CACHE["nc"] = build_program()
    return _NC_CACHE["nc"]


def _make_in_maps(x, Wq, bq, Wk, bk, Wv, bv):
    x = np.ascontiguousarray(x, dtype=np.float32)
    eye = np.eye(P, dtype=np.float32)
    aug_cols = np.zeros((P, 2), dtype=np.float32)
    aug_cols[:, 0] = 1.0
    ones_row = np.ones((1, P), dtype=ml_dtypes.bfloat16)
    bk_row = np.zeros((2, D), dtype=np.float32)
    bk_row[0] = bk.astype(np.float32)
    bv_cols = np.ascontiguousarray(
        bv.astype(ml_dtypes.bfloat16).reshape(ND, P).T)
    wk_c = np.ascontiguousarray(Wk, dtype=np.float32)
    wv_c = np.ascontiguousarray(Wv, dtype=ml_dtypes.bfloat16)
    in_maps = []
    for i in range(N_CORES):
        bq2 = np.zeros((2, P), dtype=np.float32)
        bq2[0] = bq[i * P:(i + 1) * P].astype(np.float32)
        in_maps.append({
            "x_sh": x[i * S_SH:(i + 1) * S_SH],
            "wq_sh": np.ascontiguousarray(Wq[i * P:(i + 1) * P],
                                          dtype=np.float32),
            "bq_row": bq2,
            "wk": wk_c, "bk_row": bk_row,
            "wv_b": wv_c, "bv_cols": bv_cols,
            "ident": eye, "aug_cols": aug_cols, "ones_row": ones_row,
        })
    return in_maps


def run(x, Wq, bq, Wk, bk, Wv, bv, **run_kwargs):
    nc = _get_program()
    in_maps = _make_in_maps(x, Wq, bq, Wk, bk, Wv, bv)
    res = run_bass_kernel_spmd(nc, in_maps, core_ids=list(range(N_CORES)),
                               **run_kwargs)
    out = np.concatenate([res.results[i]["out_sh"] for i in range(N_CORES)],
                         axis=0)
    attn = np.concatenate([res.results[i]["attn_sh"] for i in range(N_CORES)],
                          axis=0)
    return (out, attn), res


def kernel(x, Wq, bq, Wk, bk, Wv, bv):
    (out, attn), _ = run(x, Wq, bq, Wk, bk, Wv, bv)
    return out, attn


if __name__ == "__main__":
    rng = np.random.default_rng(0)
    x = rng.standard_normal((S, D), dtype=np.float32)
    stdv = 1.0 / np.sqrt(D)
    mk = lambda *s: rng.uniform(-stdv, stdv, s).astype(np.float32)
    out, attn = kernel(x, mk(D, D), mk(D), mk(D, D), mk(D), mk(D, D), mk(D))
    print(out.shape, attn.shape)


# revision 11
# speedup vs baseline: 1.1298x; 1.1298x over previous
"""BertAttention (feature-axis scores) Trainium2 Bass kernel, 8-core SPMD.

Reference computation (S=16384, D=1024, fp32):
    q = x @ Wq.T + bq ; k = x @ Wk.T + bk ; v = x @ Wv.T + bv
    scores = einsum('sd,se->de', q, k) / sqrt(D)        # [D, D]
    attn = softmax(scores, axis=-1)
    out = v @ attn                                       # [S, D]
    return out, attn

Restructuring (biases folded via augmented matrices):
    x~ = [x | 1 | 0]                  [S, D+2]  (zero col keeps f32r N even)
    W~q = [Wq | bq | 0], W~k likewise
    G~ = x~^T x~                      [D+2, D+2]  (seq contraction -> no x transpose)
    scores = W~q G~ W~k^T / sqrt(D)
    attn = softmax_rows(scores)
    out = x~ @ (W~v^T attn)   with W~v = [Wv | bv] -> last row = bv^T attn

G~ is symmetric: only upper-triangle column blocks are computed and
AllReduced (packed, split into two collectives so the first half of the
reduce overlaps the second half of the compute); mirror blocks are
reconstructed on-chip by PE transpose.

The scores/attn path runs in float32r (full fp32 inputs, ~1e-4 matmul
rounding) so the attn output stays accurate.  The post-softmax path
(attn AllGather, W2 = W~v^T attn, out = x~ @ W2~) runs in bf16, which
halves the gather payload and the x^T SBUF footprint.

Sharding: core i owns x rows [2048*i, 2048*(i+1)) and scores/attn rows
[128*i, 128*(i+1)).
"""
import sys
import numpy as np
import ml_dtypes

if "/opt/trn_rl_repo" not in sys.path:
    sys.path.insert(0, "/opt/trn_rl_repo")

import concourse.bass as bass
import concourse.mybir as mybir
import concourse.tile as tile
from concourse import bacc
from concourse.bass_utils import run_bass_kernel_spmd


def _install_ntff_hook_shim():
    """The agent image's antenv lacks axon_hooks; provide it so trace=True
    (NTFF profiling) works through run_bass_kernel_spmd."""
    import types
    if "antenv.axon_hooks" in sys.modules:
        return
    mod = types.ModuleType("antenv.axon_hooks")
    mod._hook = None
    mod.set_axon_ntff_profile_hook = lambda h: setattr(mod, "_hook", h)
    mod.get_axon_ntff_profile_hook = lambda: mod._hook
    sys.modules["antenv.axon_hooks"] = mod
    try:
        sys.path.insert(0, "/root/.axon_site")
        from trn_agent_boot.trn_boot import _ntff_profile_via_ctypes
        mod._hook = _ntff_profile_via_ctypes("/opt/axon/libaxon_pjrt.so")
    except Exception:
        pass


_install_ntff_hook_shim()

N_CORES = 8
S, D = 16384, 1024
S_SH = S // N_CORES          # 2048 seq rows per core
P = 128                      # partitions
NS = S_SH // P               # 16 seq chunks per core
ND = D // P                  # 8 feature chunks
H = 512                      # matmul free-dim half
DA = D + 2                   # augmented dim
f32 = mybir.dt.float32
f32r = mybir.dt.float32r
bf16 = mybir.dt.bfloat16
f16 = mybir.dt.float16
AX = mybir.AxisListType.X
ADD = mybir.AluOpType.add
MAX = mybir.AluOpType.max
EXP = mybir.ActivationFunctionType.Exp
SCALE = 1.0 / np.sqrt(np.float32(D))   # 1/32

# symmetric-G packing: per 128-row block bt, the column groups in
# GBLK[bt] (col0, width) are computed/shipped; the rest is mirrored.
GBLK = {
    0: [(0, 512), (512, 512)],
    1: [(0, 512), (512, 512)],
    2: [(256, 256), (512, 512)],
    3: [(256, 256), (512, 512)],
    4: [(512, 512)],
    5: [(512, 512)],
    6: [(768, 256)],
    7: [(768, 256)],
}
C0 = [blks[0][0] for bt, blks in sorted(GBLK.items())]
WID = [D - c for c in C0]
OFF = np.cumsum([0] + [P * w for w in WID]).tolist()
SIZE_A = OFF[4]                       # packed bytes (floats) of bt 0..3
AUG_OFF_B = OFF[8] - SIZE_A           # aug row offset inside buffer B
SIZE_B = AUG_OFF_B + 2 * DA


def build_program():
    nc = bacc.Bacc("TRN2", target_bir_lowering=False, debug=False,
                   num_devices=N_CORES)

    x_sh = nc.dram_tensor("x_sh", [S_SH, D], f32r, kind="ExternalInput").ap()
    wq_sh = nc.dram_tensor("wq_sh", [P, D], f16, kind="ExternalInput").ap()
    bq_row = nc.dram_tensor("bq_row", [2, P], f16, kind="ExternalInput").ap()
    wk = nc.dram_tensor("wk", [D, D], f16, kind="ExternalInput").ap()
    bk_row = nc.dram_tensor("bk_row", [2, D], f16, kind="ExternalInput").ap()
    wv_b = nc.dram_tensor("wv_b", [D, D], bf16, kind="ExternalInput").ap()
    bv_cols = nc.dram_tensor("bv_cols", [P, ND], bf16, kind="ExternalInput").ap()
    ident_d = nc.dram_tensor("ident", [P, P], f32r, kind="ExternalInput").ap()
    ident_h_d = nc.dram_tensor("ident_h", [P, P], f16, kind="ExternalInput").ap()
    aug_cols_d = nc.dram_tensor("aug_cols", [P, 2], f32r, kind="ExternalInput").ap()
    ones_row_d = nc.dram_tensor("ones_row", [1, P], bf16, kind="ExternalInput").ap()

    out_sh = nc.dram_tensor("out_sh", [S_SH, D], f32, kind="ExternalOutput").ap()
    attn_sh = nc.dram_tensor("attn_sh", [P, D], f32, kind="ExternalOutput").ap()

    RG = [list(range(N_CORES))]

    with tile.TileContext(nc) as tc:
        with tc.tile_pool(name="misc", bufs=1) as misc, \
             tc.tile_pool(name="dram", bufs=1, space="DRAM") as dram:
            ident = misc.tile([P, P], f32r)
            ident_h = misc.tile([P, P], f16)
            aug_cols = misc.tile([P, 2], f32r)
            ones_row = misc.tile([1, P], bf16)
            bq_sb = misc.tile([2, P], f16)
            bk_sb = misc.tile([2, D], f16)
            nc.sync.dma_start(ident[:], ident_d[:])
            nc.sync.dma_start(ident_h[:], ident_h_d[:])
            nc.sync.dma_start(aug_cols[:], aug_cols_d[:])
            nc.sync.dma_start(ones_row[:], ones_row_d[:])
            nc.sync.dma_start(bq_sb[:], bq_row[:])
            nc.sync.dma_start(bk_sb[:], bk_row[:])
            # touch the ACT exp table early so its load is off the
            # softmax critical path
            warm = misc.tile([1, 2], f32)
            nc.any.memset(warm[:], 0.0)
            nc.scalar.activation(warm[:], warm[:], EXP)

            gar_a = dram.tile([SIZE_A], f16)
            gar_ao = dram.tile([SIZE_A], f16, addr_space="Shared")
            gar_b = dram.tile([SIZE_B], f16)
            gar_bo = dram.tile([SIZE_B], f16, addr_space="Shared")
            ag_in = [dram.tile([P, H], bf16, name=f"agi{h}") for h in range(2)]
            ag_out = [dram.tile([D, H], bf16, name=f"ago{h}",
                                addr_space="Shared") for h in range(2)]

            def gpack(buf, bt, c0, w):
                base = OFF[bt] if bt < 4 else OFF[bt] - SIZE_A
                sl = buf[base:base + P * WID[bt]] \
                    .rearrange("(p w) -> p w", w=WID[bt])
                return sl[:, c0 - C0[bt]:c0 - C0[bt] + w]

            with tc.tile_pool(name="wv_pool", bufs=1) as wv_pool, \
                 tc.tile_pool(name="xT_pool", bufs=1) as xT_pool:
                wv_sb = wv_pool.tile([P, ND, D], bf16)
                bv_sb = wv_pool.tile([P, ND], bf16)
                xT = xT_pool.tile([P, ND, S_SH], bf16)   # x^T[e, s]

                with tc.tile_pool(name="xhi_pool", bufs=1) as xhi_pool:
                    x_hi = xhi_pool.tile([P, NS // 2, D], f32r)

                    # ============ Phase 1 ============
                    with tc.tile_pool(name="wkT_pool", bufs=1) as wkT_pool, \
                         tc.tile_pool(name="wq_pool", bufs=1) as wq_pool:
                        wkT = wkT_pool.tile([P, ND, D], f16)   # Wk^T[b, e]
                        wq_sb = wq_pool.tile([P, D], f16)
                        wqT = wq_pool.tile([P, ND, P], f16)    # Wq_i^T[a, d]

                        with tc.tile_pool(name="xlo_pool", bufs=1) as xlo_pool:
                            x_lo = xlo_pool.tile([P, NS // 2, D], f32r)
                            xr = x_sh.rearrange("(n p) e -> p n e", p=P)

                            def xk(ks):
                                return (x_lo[:, ks, :] if ks < NS // 2
                                        else x_hi[:, ks - NS // 2, :])

                            for ks in range(NS):
                                eng = nc.sync if ks % 2 == 0 else nc.scalar
                                eng.dma_start(xk(ks), xr[:, ks, :])
                            nc.sync.dma_start(wq_sb[:], wq_sh[:])
                            nc.sync.dma_start(
                                wv_sb[:],
                                wv_b.rearrange("(ct p) e -> p ct e", p=P))
                            nc.sync.dma_start(bv_sb[:], bv_cols[:])

                            # ---- G~ upper blocks ----
                            with tc.tile_pool(name="gstage", bufs=3) as gstage, \
                                 tc.tile_pool(name="psum_g5", bufs=4,
                                              space="PSUM") as psum_g5, \
                                 tc.tile_pool(name="psum_g2", bufs=2,
                                              space="PSUM") as psum_g2, \
                                 tc.tile_pool(name="psum_ga", bufs=2,
                                              space="PSUM") as psum_ga:
                                def g_block(bt, buf):
                                    pss = []
                                    for (c0, w) in GBLK[bt]:
                                        pool = psum_g5 if w == H else psum_g2
                                        pss.append(pool.tile(
                                            [P, w], f32, name="gps",
                                            tag=f"g{w}"))
                                    for ks in range(NS):
                                        for gi, (c0, w) in enumerate(GBLK[bt]):
                                            nc.tensor.matmul(
                                                pss[gi][:],
                                                xk(ks)[:, bt * P:(bt + 1) * P],
                                                xk(ks)[:, c0:c0 + w],
                                                start=(ks == 0),
                                                stop=(ks == NS - 1))
                                    for gi, (c0, w) in enumerate(GBLK[bt]):
                                        st = gstage.tile([P, H], f16,
                                                         name="gst", tag="gst")
                                        nc.vector.tensor_copy(st[:, :w], pss[gi][:])
                                        nc.scalar.dma_start(
                                            gpack(buf, bt, c0, w), st[:, :w])

                                for bt in range(4):
                                    g_block(bt, gar_a)

                                # ---- AllReduce part A (bt 0..3) ----
                                nc.gpsimd.collective_compute(
                                    "AllReduce", ADD, replica_groups=RG,
                                    ins=[gar_a[:]], outs=[gar_ao[:]],
                                )

                                for bt in range(4, ND):
                                    g_block(bt, gar_b)

                                # aug row [2, DA] = [sx^T, S, 0; 0...]
                                stage_a = gstage.tile([2, DA], f16, name="gsta",
                                                      tag="gst")
                                for nh in range(3):
                                    n0, n1 = ((nh * H, (nh + 1) * H) if nh < 2
                                              else (D, DA))
                                    n = n1 - n0
                                    ps = psum_ga.tile([P, H], f32, name="gpa",
                                                      tag="gpa")
                                    for ks in range(NS):
                                        rhs = (xk(ks)[:, n0:n1] if nh < 2
                                               else aug_cols[:])
                                        nc.tensor.matmul(ps[:2, :n], aug_cols[:],
                                                         rhs,
                                                         start=(ks == 0),
                                                         stop=(ks == NS - 1))
                                    nc.vector.tensor_copy(stage_a[:, n0:n1],
                                                       ps[:2, :n])
                                nc.scalar.dma_start(
                                    gar_b[AUG_OFF_B:AUG_OFF_B + 2 * DA]
                                    .rearrange("(p w) -> p w", w=DA),
                                    stage_a[:])

                            # ---- AllReduce part B (bt 4..7 + aug) ----
                            ar2_inst = nc.gpsimd.collective_compute(
                                "AllReduce", ADD, replica_groups=RG,
                                ins=[gar_b[:]], outs=[gar_bo[:]],
                            )

                            # ---- under AR: transpose x (ks 0..7), Wk, Wq_i
                            with tc.tile_pool(name="wk_chunk", bufs=2) as wk_chunk, \
                                 tc.tile_pool(name="psum_t", bufs=4,
                                              space="PSUM") as psum_t:
                                for ec in range(ND):
                                    for ks in range(NS // 2):
                                        pt = psum_t.tile([P, P], f32r,
                                                         name="pt", tag="pt")
                                        ti = nc.tensor.transpose(
                                            pt[:],
                                            xk(ks)[:, ec * P:(ec + 1) * P],
                                            ident[:])
                                        tile.add_dep_helper(
                                            ar2_inst.ins, ti.ins, False,
                                            "keep PE on G until AR2 queued")
                                        nc.vector.tensor_copy(
                                            xT[:, ec, ks * P:(ks + 1) * P],
                                            pt[:])

                                for eb in range(ND):
                                    wkc = wk_chunk.tile([P, D], f16,
                                                        name="wkc")
                                    nc.sync.dma_start(
                                        wkc[:], wk[eb * P:(eb + 1) * P, :])
                                    for bt in range(ND):
                                        pt = psum_t.tile([P, P], f16,
                                                         name="pt2", tag="pth")
                                        ti = nc.tensor.transpose(
                                            pt[:], wkc[:, bt * P:(bt + 1) * P],
                                            ident_h[:])
                                        tile.add_dep_helper(
                                            ar2_inst.ins, ti.ins, False,
                                            "keep PE on G until AR2 queued")
                                        nc.vector.tensor_copy(
                                            wkT[:, bt, eb * P:(eb + 1) * P],
                                            pt[:])

                                for at in range(ND):
                                    pt = psum_t.tile([P, P], f16, name="pt3",
                                                     tag="pth")
                                    nc.tensor.transpose(
                                        pt[:], wq_sb[:, at * P:(at + 1) * P],
                                        ident_h[:])
                                    nc.vector.tensor_copy(wqT[:, at, :], pt[:])
                        # x_lo freed

                        # ---- unpack G~ + mirrors + A~ + scores + softmax ----
                        with tc.tile_pool(name="g_pool", bufs=1) as g_pool, \
                             tc.tile_pool(name="sm_pool", bufs=1) as sm_pool, \
                             tc.tile_pool(name="psum_t2", bufs=4,
                                          space="PSUM") as psum_t2, \
                             tc.tile_pool(name="psum_a", bufs=2,
                                          space="PSUM") as psum_a, \
                             tc.tile_pool(name="psum_s", bufs=1,
                                          space="PSUM") as psum_s:
                            gsb = g_pool.tile([P, ND, DA], f16)
                            grow = g_pool.tile([2, DA], f16)
                            for bt in range(ND):
                                src = gpack(gar_ao if bt < 4 else gar_bo,
                                            bt, C0[bt], WID[bt])
                                nc.scalar.dma_start(gsb[:, bt, C0[bt]:D],
                                                    src)
                            nc.scalar.dma_start(
                                grow[:],
                                gar_bo[AUG_OFF_B:AUG_OFF_B + 2 * DA]
                                .rearrange("(p w) -> p w", w=DA))
                            # mirror lower blocks: G[bt, cb] = G[cb, bt]^T
                            for bt in range(ND):
                                for cb in range(C0[bt] // P):
                                    pt = psum_t2.tile([P, P], f16, name="mir",
                                                      tag="mir")
                                    nc.tensor.transpose(
                                        pt[:], gsb[:, cb, bt * P:(bt + 1) * P],
                                        ident_h[:])
                                    nc.vector.tensor_copy(
                                        gsb[:, bt, cb * P:(cb + 1) * P], pt[:])
                            # aug cols: G[bt, 1024:1026] = grow[:, bt-range]^T
                            for bt in range(ND):
                                pt = psum_t2.tile([P, P], f16, name="mira",
                                                  tag="mir")
                                nc.tensor.transpose(
                                    pt[:, :2], grow[:2, bt * P:(bt + 1) * P],
                                    ident_h[:2, :2])
                                nc.vector.tensor_copy(gsb[:, bt, D:DA], pt[:, :2])

                            # A~ = W~q_i G~
                            A_sb = g_pool.tile([P, DA], f16)
                            for nh in range(3):
                                n0, n1 = ((nh * H, (nh + 1) * H) if nh < 2
                                          else (D, DA))
                                n = n1 - n0
                                ps = psum_a.tile([P, H], f32, name="aps",
                                                 tag="aps")
                                for ac in range(ND):
                                    nc.tensor.matmul(ps[:, :n], wqT[:, ac, :],
                                                     gsb[:, ac, n0:n1],
                                                     start=(ac == 0),
                                                     stop=False)
                                nc.tensor.matmul(ps[:, :n], bq_sb[:],
                                                 grow[:, n0:n1],
                                                 start=False, stop=True)
                                nc.vector.tensor_copy(A_sb[:, n0:n1], ps[:, :n])

                            AT = g_pool.tile([P, ND, P], f16)
                            A_last = g_pool.tile([2, P], f16)
                            for bc in range(ND):
                                pt = psum_t2.tile([P, P], f16, name="at",
                                                  tag="mir")
                                nc.tensor.transpose(
                                    pt[:], A_sb[:, bc * P:(bc + 1) * P],
                                    ident_h[:])
                                nc.vector.tensor_copy(AT[:, bc, :], pt[:])
                            pt = psum_t2.tile([P, P], f16, name="at2",
                                              tag="mir")
                            nc.tensor.transpose(pt[:2, :], A_sb[:, D:DA],
                                                ident_h[:])
                            nc.vector.tensor_copy(A_last[:], pt[:2, :])

                            # scores in one 2-bank PSUM tile for 1-pass softmax
                            scp = psum_s.tile([P, D], f32, name="scp")
                            for nh in range(2):
                                for bc in range(ND):
                                    nc.tensor.matmul(
                                        scp[:, nh * H:(nh + 1) * H],
                                        AT[:, bc, :],
                                        wkT[:, bc, nh * H:(nh + 1) * H],
                                        start=(bc == 0), stop=False)
                                nc.tensor.matmul(scp[:, nh * H:(nh + 1) * H],
                                                 A_last[:],
                                                 bk_sb[:, nh * H:(nh + 1) * H],
                                                 start=False, stop=True)

                            mxc = sm_pool.tile([P, 1], f32)
                            nc.vector.reduce_max(mxc[:], scp[:], axis=AX)
                            negm = sm_pool.tile([P, 1], f32)
                            nc.vector.tensor_scalar_mul(negm[:], mxc[:],
                                                        -float(SCALE))
                            tsum = sm_pool.tile([P, 1], f32)
                            attn_sb = sm_pool.tile([P, D], f32)
                            nc.scalar.activation(
                                attn_sb[:], scp[:], EXP, bias=negm[:, 0:1],
                                scale=float(SCALE),
                                accum_out=tsum[:, 0:1])
                            rinv = sm_pool.tile([P, 1], f32)
                            nc.vector.reciprocal(rinv[:], tsum[:])
                            nc.vector.tensor_scalar_mul(attn_sb[:], attn_sb[:],
                                                        rinv[:, 0:1])

                            nc.sync.dma_start(attn_sh[:], attn_sb[:])
                            attn_bf = sm_pool.tile([P, D], bf16)
                            nc.vector.tensor_copy(attn_bf[:], attn_sb[:])
                            for h in range(2):
                                nc.scalar.dma_start(
                                    ag_in[h][:],
                                    attn_bf[:, h * H:(h + 1) * H])

                    # ---- AllGather attn rows (bf16, split by col half) ----
                    for h in range(2):
                        nc.gpsimd.collective_compute(
                            "AllGather", mybir.AluOpType.bypass,
                            replica_groups=RG,
                            ins=[ag_in[h][:]], outs=[ag_out[h][:]],
                        )

                    # ---- under AG: transpose x (ks 8..15) ----
                    with tc.tile_pool(name="psum_t3", bufs=4,
                                      space="PSUM") as psum_t3:
                        for ec in range(ND):
                            for ks in range(NS // 2, NS):
                                pt = psum_t3.tile([P, P], f32r, name="ptl",
                                                  tag="ptl")
                                nc.tensor.transpose(
                                    pt[:],
                                    x_hi[:, ks - NS // 2,
                                         ec * P:(ec + 1) * P],
                                    ident[:])
                                nc.vector.tensor_copy(
                                    xT[:, ec, ks * P:(ks + 1) * P], pt[:])
                # x_hi freed

                # ======== Phase 2: W2~ = W~v^T attn ; out = x~ @ W2~ ========
                with tc.tile_pool(name="w2_pool", bufs=1) as w2_pool:
                    w2 = w2_pool.tile([P, ND, D], bf16)
                    w2row = w2_pool.tile([1, D], bf16)

                    with tc.tile_pool(name="aa_pool", bufs=1) as aa_pool, \
                         tc.tile_pool(name="psum_w", bufs=4,
                                      space="PSUM") as psum_w:
                        attn_all = [aa_pool.tile([P, ND, H], bf16,
                                                 name=f"aa{h}")
                                    for h in range(2)]
                        for h in range(2):
                            agr = ag_out[h][:].rearrange(
                                "(ct p) d -> p ct d", p=P)
                            nc.scalar.dma_start(attn_all[h][:], agr[:])

                        for nh in range(2):
                            for et in range(ND):
                                ps = psum_w.tile([P, H], f32, name="wps",
                                                 tag="wps")
                                for ct in range(ND):
                                    nc.tensor.matmul(
                                        ps[:],
                                        wv_sb[:, ct, et * P:(et + 1) * P],
                                        attn_all[nh][:, ct, :],
                                        start=(ct == 0), stop=(ct == ND - 1))
                                nc.vector.tensor_copy(
                                    w2[:, et, nh * H:(nh + 1) * H], ps[:])
                            ps = psum_w.tile([P, H], f32, name="wps2",
                                             tag="wps")
                            for ct in range(ND):
                                nc.tensor.matmul(
                                    ps[:1, :], bv_sb[:, ct:ct + 1],
                                    attn_all[nh][:, ct, :],
                                    start=(ct == 0), stop=(ct == ND - 1))
                            nc.vector.tensor_copy(w2row[:, nh * H:(nh + 1) * H],
                                               ps[:1, :])

                    with tc.tile_pool(name="o_pool", bufs=4) as o_pool, \
                         tc.tile_pool(name="psum_o", bufs=4,
                                      space="PSUM") as psum_o:
                        for nh in range(2):
                            for st in range(NS):
                                ps = psum_o.tile([P, H], f32, name="ops",
                                                 tag="ops")
                                for ec in range(ND):
                                    nc.tensor.matmul(
                                        ps[:],
                                        xT[:, ec, st * P:(st + 1) * P],
                                        w2[:, ec, nh * H:(nh + 1) * H],
                                        start=(ec == 0), stop=False)
                                nc.tensor.matmul(ps[:], ones_row[:],
                                                 w2row[:, nh * H:(nh + 1) * H],
                                                 start=False, stop=True)
                                ost = o_pool.tile([P, H], f32, name="ost")
                                nc.vector.tensor_copy(ost[:], ps[:])
                                nc.sync.dma_start(
                                    out_sh[st * P:(st + 1) * P,
                                           nh * H:(nh + 1) * H], ost[:])

    nc.compile()
    return nc


_NC_CACHE = {}


def _get_program():
    if "nc" not in _NC_CACHE:
        _NC_CACHE["nc"] = build_program()
    return _NC_CACHE["nc"]


def _make_in_maps(x, Wq, bq, Wk, bk, Wv, bv):
    x = np.ascontiguousarray(x, dtype=np.float32)
    eye = np.eye(P, dtype=np.float32)
    aug_cols = np.zeros((P, 2), dtype=np.float32)
    aug_cols[:, 0] = 1.0
    ones_row = np.ones((1, P), dtype=ml_dtypes.bfloat16)
    bk_row = np.zeros((2, D), dtype=np.float16)
    bk_row[0] = bk.astype(np.float16)
    bv_cols = np.ascontiguousarray(
        bv.astype(ml_dtypes.bfloat16).reshape(ND, P).T)
    wk_c = np.ascontiguousarray(Wk, dtype=np.float16)
    eye_h = np.eye(P, dtype=np.float16)
    wv_c = np.ascontiguousarray(Wv, dtype=ml_dtypes.bfloat16)
    in_maps = []
    for i in range(N_CORES):
        bq2 = np.zeros((2, P), dtype=np.float16)
        bq2[0] = bq[i * P:(i + 1) * P].astype(np.float16)
        in_maps.append({
            "x_sh": x[i * S_SH:(i + 1) * S_SH],
            "wq_sh": np.ascontiguousarray(Wq[i * P:(i + 1) * P],
                                          dtype=np.float16),
            "bq_row": bq2,
            "wk": wk_c, "bk_row": bk_row,
            "wv_b": wv_c, "bv_cols": bv_cols,
            "ident": eye, "ident_h": eye_h,
            "aug_cols": aug_cols, "ones_row": ones_row,
        })
    return in_maps


def run(x, Wq, bq, Wk, bk, Wv, bv, **run_kwargs):
    nc = _get_program()
    in_maps = _make_in_maps(x, Wq, bq, Wk, bk, Wv, bv)
    res = run_bass_kernel_spmd(nc, in_maps, core_ids=list(range(N_CORES)),
                               **run_kwargs)
    out = np.concatenate([res.results[i]["out_sh"] for i in range(N_CORES)],
                         axis=0)
    attn = np.concatenate([res.results[i]["attn_sh"] for i in range(N_CORES)],
                          axis=0)
    return (out, attn), res


def kernel(x, Wq, bq, Wk, bk, Wv, bv):
    (out, attn), _ = run(x, Wq, bq, Wk, bk, Wv, bv)
    return out, attn


if __name__ == "__main__":
    rng = np.random.default_rng(0)
    x = rng.standard_normal((S, D), dtype=np.float32)
    stdv = 1.0 / np.sqrt(D)
    mk = lambda *s: rng.uniform(-stdv, stdv, s).astype(np.float32)
    out, attn = kernel(x, mk(D, D), mk(D), mk(D, D), mk(D), mk(D, D), mk(D))
    print(out.shape, attn.shape)


# revision 12
# speedup vs baseline: 1.1372x; 1.0065x over previous
"""BertAttention (feature-axis scores) Trainium2 Bass kernel, 8-core SPMD.

Reference computation (S=16384, D=1024, fp32):
    q = x @ Wq.T + bq ; k = x @ Wk.T + bk ; v = x @ Wv.T + bv
    scores = einsum('sd,se->de', q, k) / sqrt(D)        # [D, D]
    attn = softmax(scores, axis=-1)
    out = v @ attn                                       # [S, D]
    return out, attn

Restructuring (biases folded via augmented matrices):
    x~ = [x | 1 | 0]                  [S, D+2]  (zero col keeps f32r N even)
    W~q = [Wq | bq | 0], W~k likewise
    G~ = x~^T x~                      [D+2, D+2]  (seq contraction -> no x transpose)
    scores = W~q G~ W~k^T / sqrt(D)
    attn = softmax_rows(scores)
    out = x~ @ (W~v^T attn)   with W~v = [Wv | bv] -> last row = bv^T attn

G~ is symmetric: only upper-triangle column blocks are computed and
AllReduced (packed, split into two collectives so the first half of the
reduce overlaps the second half of the compute); mirror blocks are
reconstructed on-chip by PE transpose.

The scores/attn path runs in float32r (full fp32 inputs, ~1e-4 matmul
rounding) so the attn output stays accurate.  The post-softmax path
(attn AllGather, W2 = W~v^T attn, out = x~ @ W2~) runs in bf16, which
halves the gather payload and the x^T SBUF footprint.

Sharding: core i owns x rows [2048*i, 2048*(i+1)) and scores/attn rows
[128*i, 128*(i+1)).
"""
import sys
import numpy as np
import ml_dtypes

if "/opt/trn_rl_repo" not in sys.path:
    sys.path.insert(0, "/opt/trn_rl_repo")

import concourse.bass as bass
import concourse.mybir as mybir
import concourse.tile as tile
from concourse import bacc
from concourse.bass_utils import run_bass_kernel_spmd


def _install_ntff_hook_shim():
    """The agent image's antenv lacks axon_hooks; provide it so trace=True
    (NTFF profiling) works through run_bass_kernel_spmd."""
    import types
    if "antenv.axon_hooks" in sys.modules:
        return
    mod = types.ModuleType("antenv.axon_hooks")
    mod._hook = None
    mod.set_axon_ntff_profile_hook = lambda h: setattr(mod, "_hook", h)
    mod.get_axon_ntff_profile_hook = lambda: mod._hook
    sys.modules["antenv.axon_hooks"] = mod
    try:
        sys.path.insert(0, "/root/.axon_site")
        from trn_agent_boot.trn_boot import _ntff_profile_via_ctypes
        mod._hook = _ntff_profile_via_ctypes("/opt/axon/libaxon_pjrt.so")
    except Exception:
        pass


_install_ntff_hook_shim()

N_CORES = 8
S, D = 16384, 1024
S_SH = S // N_CORES          # 2048 seq rows per core
P = 128                      # partitions
NS = S_SH // P               # 16 seq chunks per core
ND = D // P                  # 8 feature chunks
H = 512                      # matmul free-dim half
DA = D + 2                   # augmented dim
f32 = mybir.dt.float32
f32r = mybir.dt.float32r
bf16 = mybir.dt.bfloat16
f16 = mybir.dt.float16
AX = mybir.AxisListType.X
ADD = mybir.AluOpType.add
MAX = mybir.AluOpType.max
EXP = mybir.ActivationFunctionType.Exp
SCALE = 1.0 / np.sqrt(np.float32(D))   # 1/32

# symmetric-G packing: per 128-row block bt, the column groups in
# GBLK[bt] (col0, width) are computed/shipped; the rest is mirrored.
GBLK = {
    0: [(0, 512), (512, 512)],
    1: [(0, 512), (512, 512)],
    2: [(256, 256), (512, 512)],
    3: [(256, 256), (512, 512)],
    4: [(512, 512)],
    5: [(512, 512)],
    6: [(768, 256)],
    7: [(768, 256)],
}
C0 = [blks[0][0] for bt, blks in sorted(GBLK.items())]
WID = [D - c for c in C0]
OFF = np.cumsum([0] + [P * w for w in WID]).tolist()
SIZE_A = OFF[4]                       # packed bytes (floats) of bt 0..3
AUG_OFF_B = OFF[8] - SIZE_A           # aug row offset inside buffer B
SIZE_B = AUG_OFF_B + 2 * DA


def build_program():
    nc = bacc.Bacc("TRN2", target_bir_lowering=False, debug=False,
                   num_devices=N_CORES)

    x_sh = nc.dram_tensor("x_sh", [S_SH, D], f16, kind="ExternalInput").ap()
    wq_sh = nc.dram_tensor("wq_sh", [P, D], f16, kind="ExternalInput").ap()
    bq_row = nc.dram_tensor("bq_row", [2, P], f16, kind="ExternalInput").ap()
    wk = nc.dram_tensor("wk", [D, D], f16, kind="ExternalInput").ap()
    bk_row = nc.dram_tensor("bk_row", [2, D], f16, kind="ExternalInput").ap()
    wv_b = nc.dram_tensor("wv_b", [D, D], bf16, kind="ExternalInput").ap()
    bv_cols = nc.dram_tensor("bv_cols", [P, ND], bf16, kind="ExternalInput").ap()
    ident_h_d = nc.dram_tensor("ident_h", [P, P], f16, kind="ExternalInput").ap()
    aug_cols_d = nc.dram_tensor("aug_cols", [P, 2], f16, kind="ExternalInput").ap()
    ones_row_d = nc.dram_tensor("ones_row", [1, P], bf16, kind="ExternalInput").ap()

    out_sh = nc.dram_tensor("out_sh", [S_SH, D], f32, kind="ExternalOutput").ap()
    attn_sh = nc.dram_tensor("attn_sh", [P, D], f32, kind="ExternalOutput").ap()

    RG = [list(range(N_CORES))]

    with tile.TileContext(nc) as tc:
        with tc.tile_pool(name="misc", bufs=1) as misc, \
             tc.tile_pool(name="dram", bufs=1, space="DRAM") as dram:
            ident_h = misc.tile([P, P], f16)
            aug_cols = misc.tile([P, 2], f16)
            ones_row = misc.tile([1, P], bf16)
            bq_sb = misc.tile([2, P], f16)
            bk_sb = misc.tile([2, D], f16)
            nc.sync.dma_start(ident_h[:], ident_h_d[:])
            nc.sync.dma_start(aug_cols[:], aug_cols_d[:])
            nc.sync.dma_start(ones_row[:], ones_row_d[:])
            nc.sync.dma_start(bq_sb[:], bq_row[:])
            nc.sync.dma_start(bk_sb[:], bk_row[:])
            # touch the ACT exp table early so its load is off the
            # softmax critical path
            warm = misc.tile([1, 2], f32)
            nc.any.memset(warm[:], 0.0)
            nc.scalar.activation(warm[:], warm[:], EXP)

            gar_a = dram.tile([SIZE_A], f16)
            gar_ao = dram.tile([SIZE_A], f16, addr_space="Shared")
            gar_b = dram.tile([SIZE_B], f16)
            gar_bo = dram.tile([SIZE_B], f16, addr_space="Shared")
            ag_in = [dram.tile([P, H], bf16, name=f"agi{h}") for h in range(2)]
            ag_out = [dram.tile([D, H], bf16, name=f"ago{h}",
                                addr_space="Shared") for h in range(2)]

            def gpack(buf, bt, c0, w):
                base = OFF[bt] if bt < 4 else OFF[bt] - SIZE_A
                sl = buf[base:base + P * WID[bt]] \
                    .rearrange("(p w) -> p w", w=WID[bt])
                return sl[:, c0 - C0[bt]:c0 - C0[bt] + w]

            with tc.tile_pool(name="wv_pool", bufs=1) as wv_pool, \
                 tc.tile_pool(name="xT_pool", bufs=1) as xT_pool:
                wv_sb = wv_pool.tile([P, ND, D], bf16)
                bv_sb = wv_pool.tile([P, ND], bf16)
                xT = xT_pool.tile([P, ND, S_SH], bf16)   # x^T[e, s]

                with tc.tile_pool(name="xhi_pool", bufs=1) as xhi_pool:
                    x_hi = xhi_pool.tile([P, NS // 2, D], f16)

                    # ============ Phase 1 ============
                    with tc.tile_pool(name="wkT_pool", bufs=1) as wkT_pool, \
                         tc.tile_pool(name="wq_pool", bufs=1) as wq_pool:
                        wkT = wkT_pool.tile([P, ND, D], f16)   # Wk^T[b, e]
                        wq_sb = wq_pool.tile([P, D], f16)
                        wqT = wq_pool.tile([P, ND, P], f16)    # Wq_i^T[a, d]

                        with tc.tile_pool(name="xlo_pool", bufs=1) as xlo_pool:
                            x_lo = xlo_pool.tile([P, NS // 2, D], f16)
                            xr = x_sh.rearrange("(n p) e -> p n e", p=P)

                            def xk(ks):
                                return (x_lo[:, ks, :] if ks < NS // 2
                                        else x_hi[:, ks - NS // 2, :])

                            for ks in range(NS):
                                eng = nc.sync if ks % 2 == 0 else nc.scalar
                                eng.dma_start(xk(ks), xr[:, ks, :])
                            nc.sync.dma_start(wq_sb[:], wq_sh[:])
                            nc.sync.dma_start(
                                wv_sb[:],
                                wv_b.rearrange("(ct p) e -> p ct e", p=P))
                            nc.sync.dma_start(bv_sb[:], bv_cols[:])

                            # ---- G~ upper blocks ----
                            with tc.tile_pool(name="gstage", bufs=3) as gstage, \
                                 tc.tile_pool(name="psum_g5", bufs=4,
                                              space="PSUM") as psum_g5, \
                                 tc.tile_pool(name="psum_g2", bufs=2,
                                              space="PSUM") as psum_g2, \
                                 tc.tile_pool(name="psum_ga", bufs=2,
                                              space="PSUM") as psum_ga:
                                def g_block(bt, buf):
                                    pss = []
                                    for (c0, w) in GBLK[bt]:
                                        pool = psum_g5 if w == H else psum_g2
                                        pss.append(pool.tile(
                                            [P, w], f32, name="gps",
                                            tag=f"g{w}"))
                                    for ks in range(NS):
                                        for gi, (c0, w) in enumerate(GBLK[bt]):
                                            nc.tensor.matmul(
                                                pss[gi][:],
                                                xk(ks)[:, bt * P:(bt + 1) * P],
                                                xk(ks)[:, c0:c0 + w],
                                                start=(ks == 0),
                                                stop=(ks == NS - 1))
                                    for gi, (c0, w) in enumerate(GBLK[bt]):
                                        st = gstage.tile([P, H], f16,
                                                         name="gst", tag="gst")
                                        nc.vector.tensor_copy(st[:, :w], pss[gi][:])
                                        nc.scalar.dma_start(
                                            gpack(buf, bt, c0, w), st[:, :w])

                                for bt in range(4):
                                    g_block(bt, gar_a)

                                # ---- AllReduce part A (bt 0..3) ----
                                nc.gpsimd.collective_compute(
                                    "AllReduce", ADD, replica_groups=RG,
                                    ins=[gar_a[:]], outs=[gar_ao[:]],
                                )

                                for bt in range(4, ND):
                                    g_block(bt, gar_b)

                                # aug row [2, DA] = [sx^T, S, 0; 0...]
                                stage_a = gstage.tile([2, DA], f16, name="gsta",
                                                      tag="gst")
                                for nh in range(3):
                                    n0, n1 = ((nh * H, (nh + 1) * H) if nh < 2
                                              else (D, DA))
                                    n = n1 - n0
                                    ps = psum_ga.tile([P, H], f32, name="gpa",
                                                      tag="gpa")
                                    for ks in range(NS):
                                        rhs = (xk(ks)[:, n0:n1] if nh < 2
                                               else aug_cols[:])
                                        nc.tensor.matmul(ps[:2, :n], aug_cols[:],
                                                         rhs,
                                                         start=(ks == 0),
                                                         stop=(ks == NS - 1))
                                    nc.vector.tensor_copy(stage_a[:, n0:n1],
                                                       ps[:2, :n])
                                nc.scalar.dma_start(
                                    gar_b[AUG_OFF_B:AUG_OFF_B + 2 * DA]
                                    .rearrange("(p w) -> p w", w=DA),
                                    stage_a[:])

                            # ---- AllReduce part B (bt 4..7 + aug) ----
                            ar2_inst = nc.gpsimd.collective_compute(
                                "AllReduce", ADD, replica_groups=RG,
                                ins=[gar_b[:]], outs=[gar_bo[:]],
                            )

                            # ---- under AR: transpose x (ks 0..7), Wk, Wq_i
                            with tc.tile_pool(name="wk_chunk", bufs=2) as wk_chunk, \
                                 tc.tile_pool(name="psum_t", bufs=4,
                                              space="PSUM") as psum_t:
                                for ec in range(ND):
                                    for ks in range(NS // 2):
                                        pt = psum_t.tile([P, P], f16,
                                                         name="pt", tag="pt")
                                        ti = nc.tensor.transpose(
                                            pt[:],
                                            xk(ks)[:, ec * P:(ec + 1) * P],
                                            ident_h[:])
                                        tile.add_dep_helper(
                                            ar2_inst.ins, ti.ins, False,
                                            "keep PE on G until AR2 queued")
                                        nc.vector.tensor_copy(
                                            xT[:, ec, ks * P:(ks + 1) * P],
                                            pt[:])

                                for eb in range(ND):
                                    wkc = wk_chunk.tile([P, D], f16,
                                                        name="wkc")
                                    nc.sync.dma_start(
                                        wkc[:], wk[eb * P:(eb + 1) * P, :])
                                    for bt in range(ND):
                                        pt = psum_t.tile([P, P], f16,
                                                         name="pt2", tag="pth")
                                        ti = nc.tensor.transpose(
                                            pt[:], wkc[:, bt * P:(bt + 1) * P],
                                            ident_h[:])
                                        tile.add_dep_helper(
                                            ar2_inst.ins, ti.ins, False,
                                            "keep PE on G until AR2 queued")
                                        nc.vector.tensor_copy(
                                            wkT[:, bt, eb * P:(eb + 1) * P],
                                            pt[:])

                                for at in range(ND):
                                    pt = psum_t.tile([P, P], f16, name="pt3",
                                                     tag="pth")
                                    nc.tensor.transpose(
                                        pt[:], wq_sb[:, at * P:(at + 1) * P],
                                        ident_h[:])
                                    nc.vector.tensor_copy(wqT[:, at, :], pt[:])
                        # x_lo freed

                        # ---- unpack G~ + mirrors + A~ + scores + softmax ----
                        with tc.tile_pool(name="g_pool", bufs=1) as g_pool, \
                             tc.tile_pool(name="sm_pool", bufs=1) as sm_pool, \
                             tc.tile_pool(name="psum_t2", bufs=4,
                                          space="PSUM") as psum_t2, \
                             tc.tile_pool(name="psum_a", bufs=2,
                                          space="PSUM") as psum_a, \
                             tc.tile_pool(name="psum_s", bufs=1,
                                          space="PSUM") as psum_s:
                            gsb = g_pool.tile([P, ND, DA], f16)
                            grow = g_pool.tile([2, DA], f16)
                            for bt in range(ND):
                                src = gpack(gar_ao if bt < 4 else gar_bo,
                                            bt, C0[bt], WID[bt])
                                nc.scalar.dma_start(gsb[:, bt, C0[bt]:D],
                                                    src)
                            nc.scalar.dma_start(
                                grow[:],
                                gar_bo[AUG_OFF_B:AUG_OFF_B + 2 * DA]
                                .rearrange("(p w) -> p w", w=DA))
                            # mirror lower blocks: G[bt, cb] = G[cb, bt]^T
                            for bt in range(ND):
                                for cb in range(C0[bt] // P):
                                    pt = psum_t2.tile([P, P], f16, name="mir",
                                                      tag="mir")
                                    nc.tensor.transpose(
                                        pt[:], gsb[:, cb, bt * P:(bt + 1) * P],
                                        ident_h[:])
                                    nc.vector.tensor_copy(
                                        gsb[:, bt, cb * P:(cb + 1) * P], pt[:])
                            # aug cols: G[bt, 1024:1026] = grow[:, bt-range]^T
                            for bt in range(ND):
                                pt = psum_t2.tile([P, P], f16, name="mira",
                                                  tag="mir")
                                nc.tensor.transpose(
                                    pt[:, :2], grow[:2, bt * P:(bt + 1) * P],
                                    ident_h[:2, :2])
                                nc.vector.tensor_copy(gsb[:, bt, D:DA], pt[:, :2])

                            # A~ = W~q_i G~
                            A_sb = g_pool.tile([P, DA], f16)
                            for nh in range(3):
                                n0, n1 = ((nh * H, (nh + 1) * H) if nh < 2
                                          else (D, DA))
                                n = n1 - n0
                                ps = psum_a.tile([P, H], f32, name="aps",
                                                 tag="aps")
                                for ac in range(ND):
                                    nc.tensor.matmul(ps[:, :n], wqT[:, ac, :],
                                                     gsb[:, ac, n0:n1],
                                                     start=(ac == 0),
                                                     stop=False)
                                nc.tensor.matmul(ps[:, :n], bq_sb[:],
                                                 grow[:, n0:n1],
                                                 start=False, stop=True)
                                nc.vector.tensor_copy(A_sb[:, n0:n1], ps[:, :n])

                            AT = g_pool.tile([P, ND, P], f16)
                            A_last = g_pool.tile([2, P], f16)
                            for bc in range(ND):
                                pt = psum_t2.tile([P, P], f16, name="at",
                                                  tag="mir")
                                nc.tensor.transpose(
                                    pt[:], A_sb[:, bc * P:(bc + 1) * P],
                                    ident_h[:])
                                nc.vector.tensor_copy(AT[:, bc, :], pt[:])
                            pt = psum_t2.tile([P, P], f16, name="at2",
                                              tag="mir")
                            nc.tensor.transpose(pt[:2, :], A_sb[:, D:DA],
                                                ident_h[:])
                            nc.vector.tensor_copy(A_last[:], pt[:2, :])

                            # scores in one 2-bank PSUM tile for 1-pass softmax
                            scp = psum_s.tile([P, D], f32, name="scp")
                            for nh in range(2):
                                for bc in range(ND):
                                    nc.tensor.matmul(
                                        scp[:, nh * H:(nh + 1) * H],
                                        AT[:, bc, :],
                                        wkT[:, bc, nh * H:(nh + 1) * H],
                                        start=(bc == 0), stop=False)
                                nc.tensor.matmul(scp[:, nh * H:(nh + 1) * H],
                                                 A_last[:],
                                                 bk_sb[:, nh * H:(nh + 1) * H],
                                                 start=False, stop=True)

                            mxc = sm_pool.tile([P, 1], f32)
                            nc.vector.reduce_max(mxc[:], scp[:], axis=AX)
                            negm = sm_pool.tile([P, 1], f32)
                            nc.vector.tensor_scalar_mul(negm[:], mxc[:],
                                                        -float(SCALE))
                            tsum = sm_pool.tile([P, 1], f32)
                            attn_sb = sm_pool.tile([P, D], f32)
                            nc.scalar.activation(
                                attn_sb[:], scp[:], EXP, bias=negm[:, 0:1],
                                scale=float(SCALE),
                                accum_out=tsum[:, 0:1])
                            rinv = sm_pool.tile([P, 1], f32)
                            nc.vector.reciprocal(rinv[:], tsum[:])
                            nc.vector.tensor_scalar_mul(attn_sb[:], attn_sb[:],
                                                        rinv[:, 0:1])

                            nc.sync.dma_start(attn_sh[:], attn_sb[:])
                            attn_bf = sm_pool.tile([P, D], bf16)
                            nc.vector.tensor_copy(attn_bf[:], attn_sb[:])
                            for h in range(2):
                                nc.scalar.dma_start(
                                    ag_in[h][:],
                                    attn_bf[:, h * H:(h + 1) * H])

                    # ---- AllGather attn rows (bf16, split by col half) ----
                    for h in range(2):
                        nc.gpsimd.collective_compute(
                            "AllGather", mybir.AluOpType.bypass,
                            replica_groups=RG,
                            ins=[ag_in[h][:]], outs=[ag_out[h][:]],
                        )

                    # ---- under AG: transpose x (ks 8..15) ----
                    with tc.tile_pool(name="psum_t3", bufs=4,
                                      space="PSUM") as psum_t3:
                        for ec in range(ND):
                            for ks in range(NS // 2, NS):
                                pt = psum_t3.tile([P, P], f16, name="ptl",
                                                  tag="ptl")
                                nc.tensor.transpose(
                                    pt[:],
                                    x_hi[:, ks - NS // 2,
                                         ec * P:(ec + 1) * P],
                                    ident_h[:])
                                nc.vector.tensor_copy(
                                    xT[:, ec, ks * P:(ks + 1) * P], pt[:])
                # x_hi freed

                # ======== Phase 2: W2~ = W~v^T attn ; out = x~ @ W2~ ========
                with tc.tile_pool(name="w2_pool", bufs=1) as w2_pool:
                    w2 = w2_pool.tile([P, ND, D], bf16)
                    w2row = w2_pool.tile([1, D], bf16)

                    with tc.tile_pool(name="aa_pool", bufs=1) as aa_pool, \
                         tc.tile_pool(name="psum_w", bufs=4,
                                      space="PSUM") as psum_w:
                        attn_all = [aa_pool.tile([P, ND, H], bf16,
                                                 name=f"aa{h}")
                                    for h in range(2)]
                        for h in range(2):
                            agr = ag_out[h][:].rearrange(
                                "(ct p) d -> p ct d", p=P)
                            nc.scalar.dma_start(attn_all[h][:], agr[:])

                        for nh in range(2):
                            for et in range(ND):
                                ps = psum_w.tile([P, H], f32, name="wps",
                                                 tag="wps")
                                for ct in range(ND):
                                    nc.tensor.matmul(
                                        ps[:],
                                        wv_sb[:, ct, et * P:(et + 1) * P],
                                        attn_all[nh][:, ct, :],
                                        start=(ct == 0), stop=(ct == ND - 1))
                                nc.vector.tensor_copy(
                                    w2[:, et, nh * H:(nh + 1) * H], ps[:])
                            ps = psum_w.tile([P, H], f32, name="wps2",
                                             tag="wps")
                            for ct in range(ND):
                                nc.tensor.matmul(
                                    ps[:1, :], bv_sb[:, ct:ct + 1],
                                    attn_all[nh][:, ct, :],
                                    start=(ct == 0), stop=(ct == ND - 1))
                            nc.vector.tensor_copy(w2row[:, nh * H:(nh + 1) * H],
                                               ps[:1, :])

                    with tc.tile_pool(name="o_pool", bufs=4) as o_pool, \
                         tc.tile_pool(name="psum_o", bufs=4,
                                      space="PSUM") as psum_o:
                        for nh in range(2):
                            for st in range(NS):
                                ps = psum_o.tile([P, H], f32, name="ops",
                                                 tag="ops")
                                for ec in range(ND):
                                    nc.tensor.matmul(
                                        ps[:],
                                        xT[:, ec, st * P:(st + 1) * P],
                                        w2[:, ec, nh * H:(nh + 1) * H],
                                        start=(ec == 0), stop=False)
                                nc.tensor.matmul(ps[:], ones_row[:],
                                                 w2row[:, nh * H:(nh + 1) * H],
                                                 start=False, stop=True)
                                ost = o_pool.tile([P, H], f32, name="ost")
                                nc.vector.tensor_copy(ost[:], ps[:])
                                nc.sync.dma_start(
                                    out_sh[st * P:(st + 1) * P,
                                           nh * H:(nh + 1) * H], ost[:])

    nc.compile()
    return nc


_NC_CACHE = {}


def _get_program():
    if "nc" not in _NC_CACHE:
        _NC_CACHE["nc"] = build_program()
    return _NC_CACHE["nc"]


def _make_in_maps(x, Wq, bq, Wk, bk, Wv, bv):
    x = np.ascontiguousarray(x, dtype=np.float16)
    aug_cols = np.zeros((P, 2), dtype=np.float16)
    aug_cols[:, 0] = 1.0
    ones_row = np.ones((1, P), dtype=ml_dtypes.bfloat16)
    bk_row = np.zeros((2, D), dtype=np.float16)
    bk_row[0] = bk.astype(np.float16)
    bv_cols = np.ascontiguousarray(
        bv.astype(ml_dtypes.bfloat16).reshape(ND, P).T)
    wk_c = np.ascontiguousarray(Wk, dtype=np.float16)
    eye_h = np.eye(P, dtype=np.float16)
    wv_c = np.ascontiguousarray(Wv, dtype=ml_dtypes.bfloat16)
    in_maps = []
    for i in range(N_CORES):
        bq2 = np.zeros((2, P), dtype=np.float16)
        bq2[0] = bq[i * P:(i + 1) * P].astype(np.float16)
        in_maps.append({
            "x_sh": x[i * S_SH:(i + 1) * S_SH],
            "wq_sh": np.ascontiguousarray(Wq[i * P:(i + 1) * P],
                                          dtype=np.float16),
            "bq_row": bq2,
            "wk": wk_c, "bk_row": bk_row,
            "wv_b": wv_c, "bv_cols": bv_cols,
            "ident_h": eye_h,
            "aug_cols": aug_cols, "ones_row": ones_row,
        })
    return in_maps


def run(x, Wq, bq, Wk, bk, Wv, bv, **run_kwargs):
    nc = _get_program()
    in_maps = _make_in_maps(x, Wq, bq, Wk, bk, Wv, bv)
    res = run_bass_kernel_spmd(nc, in_maps, core_ids=list(range(N_CORES)),
                               **run_kwargs)
    out = np.concatenate([res.results[i]["out_sh"] for i in range(N_CORES)],
                         axis=0)
    attn = np.concatenate([res.results[i]["attn_sh"] for i in range(N_CORES)],
                          axis=0)
    return (out, attn), res


def kernel(x, Wq, bq, Wk, bk, Wv, bv):
    (out, attn), _ = run(x, Wq, bq, Wk, bk, Wv, bv)
    return out, attn


if __name__ == "__main__":
    rng = np.random.default_rng(0)
    x = rng.standard_normal((S, D), dtype=np.float32)
    stdv = 1.0 / np.sqrt(D)
    mk = lambda *s: rng.uniform(-stdv, stdv, s).astype(np.float32)
    out, attn = kernel(x, mk(D, D), mk(D), mk(D, D), mk(D), mk(D, D), mk(D))
    print(out.shape, attn.shape)
